# revision 1
# baseline (speedup 1.0000x reference)
"""Multi-head attention (B=4, S=2048, D=1024, H=16) on 8 trn2 NeuronCores.

Sharding (tensor-parallel on heads, data-parallel on batch): core c handles
batch b = c//2 and head-group g = c%2 (8 of the 16 heads, i.e. output columns
512g..512g+511 of the QKV projections and input rows 512g..512g+511 of the
out-projection).  Each core emits a PARTIAL output [2048, 1024]; the host sums
the two partials per batch during the gather (row-parallel linear reduce).

Layout strategy ("transposed scores"): all projections that feed the score
matmul are computed transposed (QT/KT = [d_head-cols, seq] with d on
partitions), so scores come out as scoresT = [k_seq on partitions, q_seq on
free].  The softmax sum over k is obtained for free by appending a
ones-column to V (M=65 context matmul); exp() is ScalarE reading PSUM
directly.  No on-chip transposes anywhere.  All matmuls in bf16 with fp32
PSUM accumulation.

Phase overlap: the attention inner loop is ScalarE(exp)-bound, so the QK
projections are emitted per head-pair and interleaved between attention
pairs — TensorE fills its exp-wait slack with projection matmuls.
"""

import numpy as np
import ml_dtypes

B, S, D = 4, 2048, 1024
H, DH = 16, 64
NCORES = 8
P = 128
KT_IN = D // P       # 8 contraction tiles for the projections
HL = H // 2          # 8 local heads per core
DL = HL * DH         # 512 local projection columns
NPAIR = HL // 2      # 4 local head pairs
MT = DL // P         # 4 m-tiles for QT/KT projections
NKT = S // P         # 16 k-tiles in the attention contraction
QCH = S // 512       # 4 q-chunks (full sequence of queries per core)
VW = DH + 1          # 65: V columns per head incl. the ones column

BF16 = ml_dtypes.bfloat16

_NC_CACHE = {}


def _build_nc(phases="all"):
    import concourse.bass as bass
    import concourse.mybir as mybir
    import concourse.tile as tile
    from concourse import bacc
    from contextlib import ExitStack

    dt = mybir.dt
    F32, BF = dt.float32, dt.bfloat16
    AF = mybir.ActivationFunctionType
    ALU = mybir.AluOpType

    nc = bacc.Bacc(None)

    qT_d = nc.dram_tensor("qT", [KT_IN, P, S], BF, kind="ExternalInput")
    kT_d = nc.dram_tensor("kT", [KT_IN, P, S], BF, kind="ExternalInput")
    vT_d = nc.dram_tensor("vT", [KT_IN, P, S], BF, kind="ExternalInput")
    wq_d = nc.dram_tensor("wq", [KT_IN, P, DL], BF, kind="ExternalInput")
    wk_d = nc.dram_tensor("wk", [KT_IN, P, DL], BF, kind="ExternalInput")
    wv_d = nc.dram_tensor("wv", [KT_IN, P, DL], BF, kind="ExternalInput")
    wo_d = nc.dram_tensor("wo", [P, NPAIR, D], BF, kind="ExternalInput")
    bq_d = nc.dram_tensor("bq", [P, MT], F32, kind="ExternalInput")
    bk_d = nc.dram_tensor("bk", [P, MT], F32, kind="ExternalInput")
    bo_d = nc.dram_tensor("bo", [P, D], BF, kind="ExternalInput")
    out_d = nc.dram_tensor("out", [S, D], F32, kind="ExternalOutput")

    with tile.TileContext(nc) as tc, ExitStack() as ctx:
        persist = ctx.enter_context(tc.tile_pool(name="persist", bufs=1))
        wpool = ctx.enter_context(tc.tile_pool(name="wpool", bufs=1))
        stream = ctx.enter_context(tc.tile_pool(name="stream", bufs=4))
        vstream = ctx.enter_context(tc.tile_pool(name="vstream", bufs=3))
        ppool = ctx.enter_context(tc.tile_pool(name="ppool", bufs=3))
        small = ctx.enter_context(tc.tile_pool(name="small", bufs=2))
        ostream = ctx.enter_context(tc.tile_pool(name="ostream", bufs=3))
        psum_pr = ctx.enter_context(
            tc.tile_pool(name="psum_pr", bufs=2, space="PSUM")
        )
        psum_sc = ctx.enter_context(
            tc.tile_pool(name="psum_sc", bufs=2, space="PSUM")
        )
        psum_cx = ctx.enter_context(
            tc.tile_pool(name="psum_cx", bufs=2, space="PSUM")
        )

        # ---- persistent SBUF tensors
        QT = persist.tile([P, MT, S], BF)          # [128(d of pair), pair, q]
        KT = persist.tile([P, MT, S], BF)          # [128(d of pair), pair, k]
        VA = persist.tile([P, NKT, HL * VW], BF)   # [128(k), s-tile, 65*hl + d]
        CT = persist.tile([P, NPAIR, S], BF)       # ctxT, pair-packed rows

        BQ = wpool.tile([P, MT], F32)
        BK = wpool.tile([P, MT], F32)
        WO = wpool.tile([P, NPAIR, D], BF)
        BO = wpool.tile([P, D], BF)
        ONES = wpool.tile([P, P], BF)
        WQ = wpool.tile([P, KT_IN, DL], BF)
        WK = wpool.tile([P, KT_IN, DL], BF)
        WV = wpool.tile([P, KT_IN, DL], BF)

        nc.sync.dma_start(WQ, wq_d[:].rearrange("k p d -> p k d"))
        nc.sync.dma_start(WK, wk_d[:].rearrange("k p d -> p k d"))
        nc.sync.dma_start(WV, wv_d[:].rearrange("k p d -> p k d"))
        nc.sync.dma_start(BQ, bq_d[:])
        nc.sync.dma_start(BK, bk_d[:])
        nc.sync.dma_start(WO, wo_d[:])
        nc.sync.dma_start(BO, bo_d[:])
        nc.vector.memset(ONES, 1.0)
        # ones columns of V_aug (col 64 of each head's 65-wide block)
        va_h = VA[:].rearrange("p t (h e) -> p t h e", e=VW)
        nc.vector.memset(va_h[:, :, :, DH : DH + 1], 1.0)

        def emit_qt_proj(j, c):
            qs = stream.tile([P, KT_IN, 512], BF, tag="qs")
            nc.sync.dma_start(
                qs,
                qT_d[:, :, c * 512 : (c + 1) * 512].rearrange("k p s -> p k s"),
            )
            ps = psum_pr.tile([P, 512], F32, tag="prps")
            for kt in range(KT_IN):
                nc.tensor.matmul(
                    ps,
                    lhsT=WQ[:, kt, j * P : (j + 1) * P],
                    rhs=qs[:, kt, :],
                    start=(kt == 0),
                    stop=(kt == KT_IN - 1),
                )
            nc.vector.tensor_tensor(
                QT[:, j, c * 512 : (c + 1) * 512],
                ps,
                BQ[:, j : j + 1].to_broadcast([P, 512]),
                ALU.add,
            )

        def emit_kt_proj(j):
            """KT projection for head-pair j (m-tile j), all chunks."""
            for c in range(QCH):
                ks = stream.tile([P, KT_IN, 512], BF, tag="ks")
                nc.sync.dma_start(
                    ks,
                    kT_d[:, :, c * 512 : (c + 1) * 512].rearrange(
                        "k p s -> p k s"
                    ),
                )
                ps2 = psum_pr.tile([P, 512], F32, tag="prps")
                for kt in range(KT_IN):
                    nc.tensor.matmul(
                        ps2,
                        lhsT=WK[:, kt, j * P : (j + 1) * P],
                        rhs=ks[:, kt, :],
                        start=(kt == 0),
                        stop=(kt == KT_IN - 1),
                    )
                nc.vector.tensor_tensor(
                    KT[:, j, c * 512 : (c + 1) * 512],
                    ps2,
                    BK[:, j : j + 1].to_broadcast([P, 512]),
                    ALU.add,
                )

        def emit_v_proj(st):
            vs = vstream.tile([P, KT_IN, P], BF, tag="vs")
            nc.sync.dma_start(
                vs, vT_d[:, :, st * P : (st + 1) * P].rearrange("k p s -> p k s")
            )
            ps = psum_pr.tile([P, 512], F32, tag="prps")
            for kt in range(KT_IN):
                nc.tensor.matmul(
                    ps,
                    lhsT=vs[:, kt, :],
                    rhs=WV[:, kt, :],
                    start=(kt == 0),
                    stop=(kt == KT_IN - 1),
                )
            dst = va_h[:, st, :, 0:DH]
            nc.vector.tensor_copy(dst, ps[:].rearrange("p (h e) -> p h e", e=DH))

        inv_sqrt_dh = 1.0 / float(np.sqrt(DH))

        def emit_attn_unit(j, c, interleave_v=False):
            for v in range(2):  # heads serial: head B hides A's normalize
                lo = 64 * v
                hl = 2 * j + v
                cx = psum_cx.tile([P, 512], F32, tag="cxps")
                for g in range(NKT // 2):  # groups of 2 k-tiles
                    if interleave_v and v == 0:
                        emit_v_proj(2 * g)
                        emit_v_proj(2 * g + 1)
                    sc = psum_sc.tile([P, 2, 512], F32, tag="scps")
                    for t in range(2):
                        kt = 2 * g + t
                        nc.tensor.matmul(
                            sc[:, t, :],
                            lhsT=KT[lo : lo + 64, j, kt * P : (kt + 1) * P],
                            rhs=QT[lo : lo + 64, j, c * 512 : (c + 1) * 512],
                            start=True,
                            stop=True,
                        )
                    pt = ppool.tile([P, 2, 512], BF, tag="pt")
                    nc.scalar.activation(pt, sc, AF.Exp, scale=inv_sqrt_dh)
                    for t in range(2):
                        kt = 2 * g + t
                        nc.tensor.matmul(
                            cx[0 : DH + 1, :],
                            lhsT=VA[:, kt, VW * hl : VW * hl + VW],
                            rhs=pt[:, t, :],
                            start=(kt == 0),
                            stop=(kt == NKT - 1),
                        )
                # normalize: ctxT = ctx_unnorm * (1/rowsum) into CT
                rec = small.tile([1, 512], F32, tag="rec")
                nc.vector.reciprocal(rec, cx[DH : DH + 1, :])
                recb = small.tile([DH, 512], F32, tag="recb")
                nc.gpsimd.partition_broadcast(recb, rec)
                if v == 0:
                    nc.vector.tensor_tensor(
                        CT[0:DH, j, c * 512 : (c + 1) * 512],
                        cx[0:DH, :],
                        recb,
                        ALU.mult,
                    )
                else:
                    stg = small.tile([DH, 512], BF, tag="stg")
                    nc.vector.tensor_tensor(stg, cx[0:DH, :], recb, ALU.mult)
                    nc.sync.dma_start(
                        CT[DH:P, j, c * 512 : (c + 1) * 512], stg
                    )

        def emit_out_chunk(c):
            """Out-projection rows for q-chunk c (needs all pairs' CT chunk c)."""
            for qt in range(4 * c, 4 * (c + 1)):
                for ec in range(2):
                    ps = psum_pr.tile([P, 512], F32, tag="prps")
                    nc.tensor.matmul(
                        ps,
                        lhsT=ONES,
                        rhs=BO[:, ec * 512 : (ec + 1) * 512],
                        start=True,
                        stop=False,
                    )
                    for j in range(NPAIR):
                        nc.tensor.matmul(
                            ps,
                            lhsT=CT[:, j, qt * P : (qt + 1) * P],
                            rhs=WO[:, j, ec * 512 : (ec + 1) * 512],
                            start=False,
                            stop=(j == NPAIR - 1),
                        )
                    ot = ostream.tile([P, 512], F32, tag="ot")
                    nc.vector.tensor_copy(ot, ps)
                    nc.sync.dma_start(
                        out_d[qt * P : (qt + 1) * P, ec * 512 : (ec + 1) * 512],
                        ot,
                    )

        # ---- emission: chunk-outer; projections just-in-time; V interleaved
        # into the very first attention unit; out-projection per chunk so it
        # fills TensorE slack during the next chunk instead of a serial tail.
        if phases in ("all", "attn"):
            for c in range(QCH):
                for j in range(NPAIR):
                    if c == 0:
                        if j == 0:
                            emit_qt_proj(0, 0)
                        emit_kt_proj(j)
                        if j > 0:
                            emit_qt_proj(j, c)
                    else:
                        emit_qt_proj(j, c)
                    emit_attn_unit(j, c, interleave_v=(c == 0 and j == 0))
                    # previous chunk's out-projection AFTER the first unit of
                    # this chunk: it becomes TensorE filler under the exps
                    # instead of starving ScalarE at the chunk boundary.
                    if phases == "all" and j == 0 and c > 0:
                        emit_out_chunk(c - 1)
            if phases == "all":
                emit_out_chunk(QCH - 1)

        if phases != "all":
            dbg = ctx.enter_context(tc.tile_pool(name="dbg", bufs=2))
            for st in range(NKT):
                emit_v_proj(st)
            for j in range(NPAIR):
                emit_kt_proj(j)
            for mt in range(MT):
                dt_ = dbg.tile([P, S], F32, tag="dbg")
                nc.vector.tensor_copy(dt_, QT[:, mt, :])
                nc.sync.dma_start(out_d[mt * P : (mt + 1) * P, :], dt_)

    nc.compile()
    return nc


def _get_nc():
    if "nc" not in _NC_CACHE:
        import os

        _NC_CACHE["nc"] = _build_nc(os.environ.get("KERNEL_PHASES", "all"))
    return _NC_CACHE["nc"]


def kernel(query, key, value, Wq, bq, Wk, bk, Wv, bv, Wo, bo):
    from concourse.bass_utils import run_bass_kernel_spmd

    query = np.asarray(query, dtype=np.float32)
    key = np.asarray(key, dtype=np.float32)
    value = np.asarray(value, dtype=np.float32)
    Wq = np.asarray(Wq, dtype=np.float32)
    Wk = np.asarray(Wk, dtype=np.float32)
    Wv = np.asarray(Wv, dtype=np.float32)
    Wo = np.asarray(Wo, dtype=np.float32)
    bq = np.asarray(bq, dtype=np.float32)
    bk = np.asarray(bk, dtype=np.float32)
    bv = np.asarray(bv, dtype=np.float32)
    bo = np.asarray(bo, dtype=np.float32)

    nc = _get_nc()

    # per-head-group weight shards
    shards = []
    for g in range(2):
        cols = slice(DL * g, DL * (g + 1))
        wq_t = np.ascontiguousarray(
            Wq.reshape(KT_IN, P, D)[:, :, cols]).astype(BF16)
        wk_t = np.ascontiguousarray(
            Wk.reshape(KT_IN, P, D)[:, :, cols]).astype(BF16)
        wv_t = np.ascontiguousarray(
            Wv.reshape(KT_IN, P, D)[:, :, cols]).astype(BF16)
        # Wo rows (hl*64+d) of this group -> [ (v,d)=128, local pair j, e ]
        wo_p = np.ascontiguousarray(
            Wo[cols, :].reshape(NPAIR, 2, DH, D).transpose(1, 2, 0, 3)
            .reshape(P, NPAIR, D)).astype(BF16)
        bq_t = np.ascontiguousarray(bq[cols].reshape(MT, P).T).astype(np.float32)
        bk_t = np.ascontiguousarray(bk[cols].reshape(MT, P).T).astype(np.float32)
        # attn rows sum to 1 => this group's V bias contributes bv_g @ Wo_g;
        # the global bo is added by the g=0 core only (host sums partials).
        bo_eff = bv[cols].astype(np.float64) @ Wo[cols, :].astype(np.float64)
        if g == 0:
            bo_eff = bo_eff + bo.astype(np.float64)
        bo_rep = np.tile((bo_eff / P)[None, :], (P, 1)).astype(BF16)
        shards.append({
            "wq": wq_t, "wk": wk_t, "wv": wv_t, "wo": wo_p,
            "bq": bq_t, "bk": bk_t, "bo": bo_rep,
        })

    in_maps = []
    per_batch = {}
    for c in range(NCORES):
        b, g = divmod(c, 2)
        if b not in per_batch:
            per_batch[b] = {
                "qT": np.ascontiguousarray(query[b].T).reshape(
                    KT_IN, P, S).astype(BF16),
                "kT": np.ascontiguousarray(key[b].T).reshape(
                    KT_IN, P, S).astype(BF16),
                "vT": np.ascontiguousarray(value[b].T).reshape(
                    KT_IN, P, S).astype(BF16),
            }
        in_maps.append({**shards[g], **per_batch[b]})

    _NC_CACHE["last_in_maps"] = in_maps
    globals()["_LAST_IN_MAPS"] = in_maps
    res = run_bass_kernel_spmd(nc, in_maps, core_ids=list(range(NCORES)))

    out = np.empty((B, S, D), np.float32)
    for b in range(B):
        out[b] = res.results[2 * b]["out"] + res.results[2 * b + 1]["out"]
    return out

